# revision 1
# baseline (speedup 1.0000x reference)
"""Trainium2 Bass kernel for nn_DripBlock: per-sample modulated 3x3 conv.

Math (per sample b):
  s = w @ (linear_w / sqrt(WDIM)).T + linear_b                  [b, in_c]
  base_w = conv_w / sqrt(in_c*3*3)
  wmod = base_w * s[:,None,:,None,None]
  sigma_inv = rsqrt(sum(wmod^2, (in,ky,kx)) + 1e-8)             [b, out]
  y = conv2d(x, wmod*sigma_inv, SAME) + scale_noise*noise + bias
  out = leaky_relu(y, 0.2)

Kernel strategy (data-parallel over batch, 2 samples/core on 8 cores):
  - Fold s into x (xs = x*s per channel); conv against raw conv_w; fold
    C1*sigma_inv, bias, noise into the post ops.
  - 1D Winograd F(2,3) along W: for each output column pair (2k,2k+1),
    y_even = m1+m2+m3, y_odd = m2-m3-m4 with m_xi = V_xi . U_xi summed
    over (ic, ky).  V planes are +-1 combinations of xs columns (DVE),
    U planes are {g0, (g0+g1+g2)/2, (g0-g1+g2)/2, g2} per ky (DVE,
    one-time).  PE matmul count drops 72->48 per [oc=128 x 16rows]
    group: 4 psum planes M[xi] [128, 16*32] each accumulate 12 bf16
    matmuls (3 ky x 4 ic-chunks), N=512.
  - Inverse transform + post split across engines: ScalarE copies
    M1..M3 to SBUF bf16 with C1*sigma_inv folded into the copy
    (per-partition activation scale, halved for the U1/U2 planes);
    DVE computes t1 = M0*sig + m1 and noise+bias "pre"; GpSimd does
    the even/odd combines and pre-add (contiguous half-split layout);
    DVE applies leaky-relu while interleaving back to row-major cols.
  - x staged per 16-row band as [128, 18, 64] f32, scaled by s in
    place, V computed per band (reused across the 4 oc chunks).
  - sigma2[b,oc] = C1^2 * sum_{ic,tap} conv_w^2 s^2: ScalarE squares
    the resident bf16 weights; 36 tiny PE matmuls per oc-chunk against
    bf16 s^2 accumulate sigma2 -- scheduled after each chunk's conv.
  - conv_w loaded [oc, ic*9] (contiguous) on the scalar DMA queue, cast
    to tap-major bf16 on ScalarE, then one batched xbar DMA-transpose
    per (oc,ic) chunk gives wchunk[ic, tap, oc]; U0/U3 are views into
    it, U1/U2 are two extra bf16 tiles per chunk.
"""
import numpy as np
from math import sqrt
from contextlib import ExitStack

import concourse.bass as bass
import concourse.bacc as bacc
import concourse.mybir as mybir
import concourse.tile as tile
from concourse.masks import make_identity

B, CIN, COUT, H, W, WDIM, KK = 16, 512, 512, 64, 64, 512, 3
NCORES = 8
BLOC = B // NCORES          # 2 samples per core
P = 128
NIC = CIN // P              # 4 ic chunks
NOC = COUT // P             # 4 oc chunks
NDC = WDIM // P             # 4 wdim chunks
NBAND = 4                   # 16-row bands per sample
RB = H // NBAND             # 16 rows per band
WT = W // 2                 # 32 column tiles (2 output cols each)
XR = RB + 2                 # 18 staged rows per band
EPS = 1e-8
C0 = 1.0 / sqrt(WDIM)
C1 = 1.0 / sqrt(CIN * KK * KK)
SLOPE = 0.2

F32 = mybir.dt.float32
BF16 = mybir.dt.bfloat16
MUL = mybir.AluOpType.mult
ADD = mybir.AluOpType.add
SUB = mybir.AluOpType.subtract
MAX = mybir.AluOpType.max
AXX = mybir.AxisListType.X
COPYF = mybir.ActivationFunctionType.Copy
LRELU = mybir.ActivationFunctionType.Lrelu


def build_nc():
    nc = bacc.Bacc()

    x_d = nc.declare_dram_parameter("x", [BLOC, CIN, H, W], F32, isOutput=False)
    w_d = nc.declare_dram_parameter("w", [BLOC, WDIM], F32, isOutput=False)
    noise_d = nc.declare_dram_parameter("noise", [BLOC, 1, H, W], F32, isOutput=False)
    lw_d = nc.declare_dram_parameter("linear_w", [CIN, WDIM], F32, isOutput=False)
    lb_d = nc.declare_dram_parameter("linear_b", [CIN], F32, isOutput=False)
    cw_d = nc.declare_dram_parameter("conv_w", [COUT, CIN, KK, KK], F32, isOutput=False)
    sn_d = nc.declare_dram_parameter("scale_noise", [COUT], F32, isOutput=False)
    bias_d = nc.declare_dram_parameter("bias", [COUT], F32, isOutput=False)
    out_d = nc.declare_dram_parameter("out", [BLOC, COUT, H, W], F32, isOutput=True)

    with ExitStack() as ctx:
        tc = ctx.enter_context(tile.TileContext(nc))
        consts = ctx.enter_context(tc.tile_pool(name="consts", bufs=1))
        lw_pool = ctx.enter_context(tc.tile_pool(name="lw", bufs=2))
        lwt_pool = ctx.enter_context(tc.tile_pool(name="lwt", bufs=16))
        g_pool = ctx.enter_context(tc.tile_pool(name="g", bufs=5))
        co_pool = ctx.enter_context(tc.tile_pool(name="co", bufs=2))
        wt_pool = ctx.enter_context(tc.tile_pool(name="wt", bufs=NIC * NOC))
        u_pool = ctx.enter_context(tc.tile_pool(name="u", bufs=NIC * NOC))
        ut_pool = ctx.enter_context(tc.tile_pool(name="ut", bufs=2))
        small = ctx.enter_context(tc.tile_pool(name="small", bufs=1))
        nb_pool = ctx.enter_context(tc.tile_pool(name="nb", bufs=3))
        cobf_pool = ctx.enter_context(tc.tile_pool(name="cobf", bufs=2))
        xt_pool = ctx.enter_context(tc.tile_pool(name="xt", bufs=2))
        v_pool = ctx.enter_context(tc.tile_pool(name="v", bufs=2))
        m_pool = ctx.enter_context(tc.tile_pool(name="m", bufs=2))
        out_pool = ctx.enter_context(tc.tile_pool(name="out", bufs=3))

        mm_psum = ctx.enter_context(tc.tile_pool(name="mmps", bufs=8, space="PSUM"))

        # ---- constants ----
        ident = consts.tile([P, P], F32)
        make_identity(nc, ident)
        lb_cols = consts.tile([P, NIC], F32)
        nc.sync.dma_start(out=lb_cols, in_=lb_d[:].rearrange("(c p) -> p c", p=P))
        wcols = consts.tile([P, NDC, BLOC], F32)
        for b in range(BLOC):
            nc.sync.dma_start(out=wcols[:, :, b:b + 1],
                              in_=w_d[b].rearrange("(c p) -> p c", p=P).rearrange("p (c o) -> p c o", o=1))
        eps_col = consts.tile([P, 1], F32)
        nc.vector.memset(eps_col, EPS)
        bias_cols = consts.tile([P, NOC], F32)
        nc.sync.dma_start(out=bias_cols, in_=bias_d[:].rearrange("(c p) -> p c", p=P))
        sn_cols = consts.tile([P, NOC], F32)
        nc.sync.dma_start(out=sn_cols, in_=sn_d[:].rearrange("(c p) -> p c", p=P))

        # ---- phase A: s = w @ (linear_w*C0).T + linear_b, as sT[ic, b] ----
        lwt = {}
        for icc in range(NIC):
            lw_sb = lw_pool.tile([P, WDIM], F32, tag="lw")
            nc.scalar.dma_start(out=lw_sb, in_=lw_d[icc * P:(icc + 1) * P, :])
            for dc in range(NDC):
                tp = mm_psum.tile([P, P], F32, tag="mm")
                nc.tensor.transpose(tp, lw_sb[:, dc * P:(dc + 1) * P], ident)
                t = lwt_pool.tile([P, P], F32, tag="lwt")
                nc.vector.tensor_copy(out=t, in_=tp)
                lwt[(dc, icc)] = t

        sT = []
        s2T = []
        for icc in range(NIC):
            sp = mm_psum.tile([P, BLOC], F32, tag="mm")
            for dc in range(NDC):
                nc.tensor.matmul(sp, lwt[(dc, icc)], wcols[:, dc, :],
                                 start=(dc == 0), stop=(dc == NDC - 1))
            st = small.tile([P, BLOC], F32, tag=f"sT{icc}")
            nc.vector.tensor_scalar(out=st, in0=sp, scalar1=C0, scalar2=lb_cols[:, icc:icc + 1],
                                    op0=MUL, op1=ADD)
            s2 = small.tile([P, BLOC], BF16, tag=f"s2T{icc}")
            nc.vector.tensor_mul(s2, st, st)
            sT.append(st)
            s2T.append(s2)

        # ---- staging: x band -> scaled f32 xt -> V planes (bf16) ----
        vts = {}     # (b, band) -> [vt per icc]
        nrows = {}   # (b, band) -> [128, RB*W] f32 noise broadcast

        def stage_band(b, band):
            r0 = band * RB
            vt_l = []
            for icc in range(NIC):
                xt = xt_pool.tile([P, XR, W], F32, tag="xt", bufs=3)
                # xt row j holds x row (r0-1+j); rows outside [0,H) are zero
                lo = max(r0 - 1, 0)
                hi = min(r0 - 1 + XR, H)
                j0 = lo - (r0 - 1)
                if j0 > 0:
                    nc.vector.memset(xt[:, 0:j0, :], 0.0)
                if (r0 - 1 + XR) > H:
                    nc.vector.memset(xt[:, XR - 1:XR, :], 0.0)
                nc.sync.dma_start(
                    out=xt[:, j0:j0 + (hi - lo), :],
                    in_=x_d[b, icc * P:(icc + 1) * P, lo:hi, :])
                # scale by s in place (ScalarE, off the DVE)
                nc.scalar.activation(out=xt, in_=xt, func=COPYF,
                                     scale=sT[icc][:, b:b + 1])
                # V planes: [128, 4, XR, WT] bf16 (on GpSimd, off the DVE)
                vt = v_pool.tile([P, 4, XR, WT], BF16, tag=f"v{icc}")
                xte = xt.rearrange("p r (w two) -> p r w two", two=2)
                ev = xte[:, :, :, 0]   # x cols 0,2,..,62
                od = xte[:, :, :, 1]   # x cols 1,3,..,63
                # V0[k] = d(2k-1) - d(2k+1);  k=0: -x[1]
                nc.vector.tensor_tensor(out=vt[:, 0, :, 1:WT], in0=od[:, :, 0:WT - 1],
                                        in1=od[:, :, 1:WT], op=SUB)
                nc.gpsimd.tensor_scalar_mul(out=vt[:, 0, :, 0:1], in0=od[:, :, 0:1],
                                            scalar1=-1.0)
                # V1[k] = d(2k) + d(2k+1)
                nc.vector.tensor_tensor(out=vt[:, 1], in0=ev, in1=od, op=ADD)
                # V2[k] = d(2k+1) - d(2k)
                nc.vector.tensor_tensor(out=vt[:, 2], in0=od, in1=ev, op=SUB)
                # V3[k] = d(2k) - d(2k+2);  k=WT-1: x[62]
                nc.vector.tensor_tensor(out=vt[:, 3, :, 0:WT - 1], in0=ev[:, :, 0:WT - 1],
                                        in1=ev[:, :, 1:WT], op=SUB)
                nc.gpsimd.tensor_copy(out=vt[:, 3, :, WT - 1:WT], in_=ev[:, :, WT - 1:WT])
                vt_l.append(vt)
            vts[(b, band)] = vt_l
            # noise rows broadcast to all partitions (partition step-0 src AP)
            nb = nb_pool.tile([P, RB * W], F32, tag="nb")
            nsrc = noise_d[b].rearrange("o h w -> o (h w)")[0:1, r0 * W:(r0 + RB) * W]
            nsrc_bc = bass.AP(tensor=nsrc.tensor, offset=nsrc.offset,
                              ap=[[0, P]] + list(nsrc.ap)[1:])
            nc.sync.dma_start(out=nb, in_=nsrc_bc)
            nrows[(b, band)] = nb

        stage_band(0, 0)

        # ---- phase B: weights: wchunk[ic, tap, oc], U1/U2, G, sigma ----
        wchunks = {}
        u1s = {}
        u2s = {}
        gts = {}
        sig_scale = {}   # occ -> [P, BLOC] f32: C1*sigma_inv

        def emit_weights(occ):
            for icc in range(NIC):
                co = co_pool.tile([P, P * KK * KK], F32, tag="co")
                nc.scalar.dma_start(
                    out=co,
                    in_=cw_d[occ * P:(occ + 1) * P, icc * P:(icc + 1) * P, :, :]
                    .rearrange("o i a b -> o (i a b)"))
                # cast to bf16 in tap-major order (strided read, contiguous
                # write), then one batched xbar transpose per chunk:
                # wchunk[ic, tap, oc] = co_bf[oc, tap*128+ic]
                co_bf = cobf_pool.tile([P, KK * KK * P], BF16, tag="cobf")
                if icc < 2:
                    nc.scalar.copy(
                        out=co_bf.rearrange("o (n i) -> o n i", i=P),
                        in_=co.rearrange("o (i n) -> o n i", n=KK * KK))
                else:
                    nc.vector.tensor_copy(
                        out=co_bf.rearrange("o (n i) -> o n i", i=P),
                        in_=co.rearrange("o (i n) -> o n i", n=KK * KK))
                wchunk = wt_pool.tile([P, KK * KK, P], BF16, tag="wt")
                nc.scalar.dma_start_transpose(out=wchunk, in_=co_bf)
                wchunks[(icc, occ)] = wchunk
                # U1/U2: 0.5*(g0 +- g1 + g2) per ky over the kx taps
                wc4 = wchunk.rearrange("i (ky kx) o -> i ky kx o", kx=KK)
                w0 = wc4[:, :, 0, :]
                w1 = wc4[:, :, 1, :]
                w2 = wc4[:, :, 2, :]
                tu = ut_pool.tile([P, KK, P], BF16, tag="tu")
                nc.vector.tensor_tensor(out=tu, in0=w0, in1=w2, op=ADD)
                u1 = u_pool.tile([P, KK, P], BF16, tag="u1")
                nc.vector.tensor_tensor(out=u1, in0=tu, in1=w1, op=ADD)
                u2 = u_pool.tile([P, KK, P], BF16, tag="u2")
                nc.vector.tensor_tensor(out=u2, in0=tu, in1=w1, op=SUB)
                u1s[(icc, occ)] = u1
                u2s[(icc, occ)] = u2


        def emit_sigma(occ):
            sqws = []
            for icc in range(NIC):
                sqw = cobf_pool.tile([P, KK * KK, P], BF16, tag="sqw", bufs=4)
                nc.scalar.activation(out=sqw, in_=wchunks[(icc, occ)],
                                     func=mybir.ActivationFunctionType.Square)
                sqws.append(sqw)
            sg = mm_psum.tile([P, BLOC], F32, tag="mm")
            for icc in range(NIC):
                for t in range(KK * KK):
                    nc.tensor.matmul(sg, sqws[icc][:, t, :], s2T[icc],
                                     start=(icc == 0 and t == 0),
                                     stop=(icc == NIC - 1 and t == KK * KK - 1))
            # sigma = sqrt(C1^2 * sig2 + EPS); sig_scale = C1 / sigma
            sig = small.tile([P, BLOC], F32, tag=f"sig{occ}")
            nc.scalar.activation(out=sig, in_=sg, func=mybir.ActivationFunctionType.Sqrt,
                                 bias=eps_col[:, 0:1], scale=C1 * C1)
            sinv = small.tile([P, BLOC], F32, tag=f"sinv{occ}")
            nc.vector.reciprocal(out=sinv, in_=sig)
            ssc = small.tile([P, BLOC], F32, tag=f"ssc{occ}")
            nc.vector.tensor_scalar_mul(out=ssc, in0=sinv, scalar1=C1)
            ssch = small.tile([P, BLOC], F32, tag=f"ssch{occ}")
            nc.vector.tensor_scalar_mul(out=ssch, in0=sinv, scalar1=C1 * 0.5)
            sig_scale[occ] = (ssc, ssch)

        emit_weights(0)
        emit_sigma(0)

        # ---- phase C: winograd conv + post ----
        # Sample 0 runs occ-major over band PAIRS so the weight pipeline for
        # occ k is only needed ~2 bands in; sample 1 runs band-major.
        out3 = out_d.rearrange("b c h w -> b c (h w)")

        def group(b, band, occ, mid_hook=None):
            vt_l = vts[(b, band)]
            nb_t = nrows[(b, band)]
            M = [mm_psum.tile([P, RB * WT], F32, tag="mm", name=f"M{xi}")
                 for xi in range(4)]
            for icc in range(NIC):
                vt = vt_l[icc]
                for ky in range(KK):
                    lhs = (wchunks[(icc, occ)][:, 3 * ky, :],
                           u1s[(icc, occ)][:, ky, :],
                           u2s[(icc, occ)][:, ky, :],
                           wchunks[(icc, occ)][:, 3 * ky + 2, :])
                    st = (icc == 0 and ky == 0)
                    sp = (icc == NIC - 1 and ky == KK - 1)
                    for xi in range(4):
                        nc.tensor.matmul(
                            M[xi], lhs[xi], vt[:, xi, ky:ky + RB, :],
                            start=st, stop=sp)
            if mid_hook is not None:
                mid_hook()
            ssc = sig_scale[occ][0][:, b:b + 1]
            ssch = sig_scale[occ][1][:, b:b + 1]
            # ScalarE: copy M1..M3 to SBUF bf16 with sig folded in
            m1 = m_pool.tile([P, RB * WT], BF16, tag="m1", bufs=3)
            nc.scalar.activation(out=m1, in_=M[1], func=COPYF, scale=ssch)
            m2 = m_pool.tile([P, RB * WT], BF16, tag="m2", bufs=3)
            nc.scalar.activation(out=m2, in_=M[2], func=COPYF, scale=ssch)
            m3 = m_pool.tile([P, RB * WT], BF16, tag="m3", bufs=3)
            nc.scalar.activation(out=m3, in_=M[3], func=COPYF, scale=ssc)
            # pre = scale_noise*noise + bias in half-split layout
            pre = out_pool.tile([P, RB, 2, WT], F32, tag="pre", bufs=2)
            nbv = nb_t.rearrange("p (r w two) -> p r w two", r=RB, two=2)
            for par in range(2):
                nc.vector.tensor_scalar(out=pre[:, :, par, :],
                                        in0=nbv[:, :, :, par],
                                        scalar1=sn_cols[:, occ:occ + 1],
                                        scalar2=bias_cols[:, occ:occ + 1],
                                        op0=MUL, op1=ADD)
            # DVE: t1 = M0*sig + m1 (PSUM read)
            t1 = m_pool.tile([P, RB * WT], BF16, tag="t1", bufs=3)
            nc.vector.scalar_tensor_tensor(out=t1, in0=M[0], scalar=ssc,
                                           in1=m1, op0=MUL, op1=ADD)
            # GpSimd: inverse combine; even cols -> z[:,:,0,:], odd -> z[:,:,1,:]
            z = out_pool.tile([P, RB, 2, WT], F32, tag="z", bufs=2)
            ceng = nc.vector if (b == BLOC - 1 and band == NBAND - 1) \
                else nc.gpsimd
            t1v = t1.rearrange("p (r w) -> p r w", w=WT)
            m2v = m2.rearrange("p (r w) -> p r w", w=WT)
            ceng.tensor_tensor(out=z[:, :, 0, :], in0=t1v, in1=m2v, op=ADD)
            vv = m_pool.tile([P, RB * WT], BF16, tag="vv", bufs=3)
            ceng.tensor_tensor(out=vv, in0=m1, in1=m2, op=SUB)
            vvv = vv.rearrange("p (r w) -> p r w", w=WT)
            m3v = m3.rearrange("p (r w) -> p r w", w=WT)
            nc.gpsimd.tensor_tensor(out=z[:, :, 1, :], in0=vvv, in1=m3v, op=SUB)
            z2 = out_pool.tile([P, RB, 2, WT], F32, tag="z2", bufs=2)
            ceng.tensor_tensor(out=z2, in0=z, in1=pre, op=ADD)
            # DVE leaky relu + interleave back to row-major cols
            zo = out_pool.tile([P, RB * W], F32, tag="zo", bufs=2)
            zov = zo.rearrange("p (r w two) -> p r w two", r=RB, two=2)
            for par in range(2):
                zi = z2[:, :, par, :]
                nc.vector.scalar_tensor_tensor(out=zov[:, :, :, par],
                                               in0=zi, scalar=SLOPE,
                                               in1=zi, op0=MUL, op1=MAX)
            nc.sync.dma_start(
                out=out3[b, occ * P:(occ + 1) * P,
                         band * RB * W:(band + 1) * RB * W],
                in_=zo)

        for b in range(BLOC):
            for band in range(NBAND):
                nb_, nband_ = (b, band + 1) if band + 1 < NBAND else (b + 1, 0)
                if nb_ < BLOC:
                    stage_band(nb_, nband_)
                for occ in range(NOC):
                    hook = None
                    if b == 0 and band == 0 and occ >= 1:
                        hook = (lambda o=occ: emit_sigma(o))
                    group(b, band, occ, mid_hook=hook)
                    if b == 0 and band == 0 and occ + 1 < NOC:
                        emit_weights(occ + 1)
                del vts[(b, band)], nrows[(b, band)]

    nc.compile()
    return nc


_NC_CACHE = None


def _get_nc():
    global _NC_CACHE
    if _NC_CACHE is None:
        _NC_CACHE = build_nc()
    return _NC_CACHE


def kernel(**inputs):
    from concourse.bass_utils import run_bass_kernel_spmd

    nc = _get_nc()
    shard_names = ("x", "w", "noise")
    in_maps = []
    for i in range(NCORES):
        m = {}
        for k, v in inputs.items():
            v = np.ascontiguousarray(np.asarray(v), dtype=np.float32)
            if k in shard_names:
                m[k] = np.ascontiguousarray(v[i * BLOC:(i + 1) * BLOC])
            else:
                m[k] = v
        in_maps.append(m)
    res = run_bass_kernel_spmd(nc, in_maps, list(range(NCORES)))
    outs = [res.results[i]["out"] for i in range(NCORES)]
    return np.concatenate(outs, axis=0).astype(np.float32)



# revision 13
# speedup vs baseline: 1.0654x; 1.0654x over previous
"""Trainium2 Bass kernel for nn_DripBlock: per-sample modulated 3x3 conv.

Math (per sample b):
  s = w @ (linear_w / sqrt(WDIM)).T + linear_b                  [b, in_c]
  base_w = conv_w / sqrt(in_c*3*3)
  wmod = base_w * s[:,None,:,None,None]
  sigma_inv = rsqrt(sum(wmod^2, (in,ky,kx)) + 1e-8)             [b, out]
  y = conv2d(x, wmod*sigma_inv, SAME) + scale_noise*noise + bias
  out = leaky_relu(y, 0.2)

Kernel strategy (data-parallel over batch, 2 samples/core on 8 cores):
  - Fold s into x (xs = x*s per channel); conv against raw conv_w; fold
    C1*sigma_inv, bias, noise into the post ops.
  - 1D Winograd F(2,3) along W; weights stored HALVED (cast scale 0.5)
    so U1=(w0+w1+w2)/2 and U2=(w0-w1+w2)/2 are plain adds of the halved
    taps; the 2x compensation for the U0/U3 planes is folded into the
    drain's scalar_tensor_tensor scalars.
  - Drains use at most one PSUM operand per instruction (PSUM has one
    DVE read port): ScalarE copies M1 to SBUF; DVE computes
    t=2*M0+m1, e=t+M2, t2=m1-M2, o=-2*M3+t2; GpSimd computes
    pre=sn*noise+bias and z2=ssc*e/o+pre; DVE applies leaky-relu while
    interleaving even/odd back to row-major; one DMA out per group.
  - sigma via tap-reduced squared weights: sqw=wchunk^2 (GpSimd),
    wsq[ic,oc]=sum_tap sqw (DVE strided reduce), then 4 tiny f32
    matmuls against s^2 per oc chunk (16 total vs 144 in v1).
  - Schedule: occ-OUTER over band pairs.  Sample0 pair0 runs
    (occ, band) = (0,0),(0,1),(1,0),... so weight emission for occ k+1
    overlaps the two groups of occ k; all conv_w DMA is issued up front
    on the scalar queue while x/transposes/outputs ride the sync queue.
"""
import numpy as np
from math import sqrt
from contextlib import ExitStack

import concourse.bass as bass
import concourse.bacc as bacc
import concourse.mybir as mybir
import concourse.tile as tile
from concourse.masks import make_identity

B, CIN, COUT, H, W, WDIM, KK = 16, 512, 512, 64, 64, 512, 3
NCORES = 8
BLOC = B // NCORES          # 2 samples per core
P = 128
NIC = CIN // P              # 4 ic chunks
NOC = COUT // P             # 4 oc chunks
NDC = WDIM // P             # 4 wdim chunks
NBAND = 4                   # 16-row bands per sample
RB = H // NBAND             # 16 rows per band
WT = W // 2                 # 32 column tiles (2 output cols each)
XR = RB + 2                 # 18 staged rows per band
EPS = 1e-8
C0 = 1.0 / sqrt(WDIM)
C1 = 1.0 / sqrt(CIN * KK * KK)
SLOPE = 0.2

F32 = mybir.dt.float32
BF16 = mybir.dt.bfloat16
MUL = mybir.AluOpType.mult
ADD = mybir.AluOpType.add
SUB = mybir.AluOpType.subtract
MAX = mybir.AluOpType.max
COPYF = mybir.ActivationFunctionType.Copy
SQRTF = mybir.ActivationFunctionType.Sqrt


def build_nc():
    nc = bacc.Bacc()

    x_d = nc.declare_dram_parameter("x", [BLOC, CIN, H, W], F32, isOutput=False)
    w_d = nc.declare_dram_parameter("w", [BLOC, WDIM], F32, isOutput=False)
    noise_d = nc.declare_dram_parameter("noise", [BLOC, 1, H, W], F32, isOutput=False)
    lw_d = nc.declare_dram_parameter("linear_w", [CIN, WDIM], F32, isOutput=False)
    lb_d = nc.declare_dram_parameter("linear_b", [CIN], F32, isOutput=False)
    cw_d = nc.declare_dram_parameter("conv_w", [COUT, CIN, KK, KK], F32, isOutput=False)
    sn_d = nc.declare_dram_parameter("scale_noise", [COUT], F32, isOutput=False)
    bias_d = nc.declare_dram_parameter("bias", [COUT], F32, isOutput=False)
    out_d = nc.declare_dram_parameter("out", [BLOC, COUT, H, W], F32, isOutput=True)

    with ExitStack() as ctx:
        tc = ctx.enter_context(tile.TileContext(nc))
        consts = ctx.enter_context(tc.tile_pool(name="consts", bufs=1))
        lw_pool = ctx.enter_context(tc.tile_pool(name="lw", bufs=2))
        lwt_pool = ctx.enter_context(tc.tile_pool(name="lwt", bufs=4))
        co_pool = ctx.enter_context(tc.tile_pool(name="co", bufs=4))
        cobf_pool = ctx.enter_context(tc.tile_pool(name="cobf", bufs=2))
        wt_pool = ctx.enter_context(tc.tile_pool(name="wt", bufs=1))
        u_pool = ctx.enter_context(tc.tile_pool(name="u", bufs=1))
        ua_pool = ctx.enter_context(tc.tile_pool(name="ua", bufs=2))
        sqw_pool = ctx.enter_context(tc.tile_pool(name="sqw", bufs=2))
        wsq_pool = ctx.enter_context(tc.tile_pool(name="wsq", bufs=1))
        small = ctx.enter_context(tc.tile_pool(name="small", bufs=1))
        xt_pool = ctx.enter_context(tc.tile_pool(name="xt", bufs=2))
        v_pool = ctx.enter_context(tc.tile_pool(name="v", bufs=1))
        nb_pool = ctx.enter_context(tc.tile_pool(name="nb", bufs=1))
        dr_pool = ctx.enter_context(tc.tile_pool(name="dr", bufs=2))
        pz_pool = ctx.enter_context(tc.tile_pool(name="pz", bufs=2))
        out_pool = ctx.enter_context(tc.tile_pool(name="out", bufs=2))

        psum = ctx.enter_context(tc.tile_pool(name="mmps", bufs=8, space="PSUM"))

        # ---- conv_w loads: occ0 up front on the scalar queue; occ k+1
        # issued at the end of emit_weights(k) (keeps slot reuse WAR deps
        # pointing at already-emitted casts) ----
        co_tiles = {}

        def load_co(occ):
            for icc in range(NIC):
                co = co_pool.tile([P, P * KK * KK], F32, tag="co",
                                  name=f"co{icc}_{occ}")
                nc.scalar.dma_start(
                    out=co,
                    in_=cw_d[occ * P:(occ + 1) * P, icc * P:(icc + 1) * P, :, :]
                    .rearrange("o i a b -> o (i a b)"))
                co_tiles[(icc, occ)] = co

        load_co(0)

        # ---- constants ----
        ident = consts.tile([P, P], F32)
        make_identity(nc, ident)
        lb_cols = consts.tile([P, NIC], F32)
        nc.scalar.dma_start(out=lb_cols, in_=lb_d[:].rearrange("(c p) -> p c", p=P))
        wcols = consts.tile([P, NDC, BLOC], F32)
        for b in range(BLOC):
            nc.scalar.dma_start(out=wcols[:, :, b:b + 1],
                                in_=w_d[b].rearrange("(c p) -> p c", p=P)
                                .rearrange("p (c o) -> p c o", o=1))
        eps_col = consts.tile([P, 1], F32)
        nc.vector.memset(eps_col, EPS)
        bias_cols = consts.tile([P, NOC], F32)
        nc.scalar.dma_start(out=bias_cols, in_=bias_d[:].rearrange("(c p) -> p c", p=P))
        sn_cols = consts.tile([P, NOC], F32)
        nc.scalar.dma_start(out=sn_cols, in_=sn_d[:].rearrange("(c p) -> p c", p=P))

        # ---- phase A: s = w @ (linear_w*C0).T + linear_b, as sT[ic, b] ----
        sT = []
        s2T = []
        for icc in range(NIC):
            lw_sb = lw_pool.tile([P, WDIM], F32, tag="lw")
            nc.scalar.dma_start(out=lw_sb, in_=lw_d[icc * P:(icc + 1) * P, :])
            lwt = []
            for dc in range(NDC):
                tp = psum.tile([P, P], F32, tag="mm", name="tp")
                nc.tensor.transpose(tp, lw_sb[:, dc * P:(dc + 1) * P], ident)
                t_ = lwt_pool.tile([P, P], F32, tag="lwt")
                nc.vector.tensor_copy(out=t_, in_=tp)
                lwt.append(t_)
            sp = psum.tile([P, BLOC], F32, tag="mm", name=f"sp{icc}")
            for dc in range(NDC):
                nc.tensor.matmul(sp, lwt[dc], wcols[:, dc, :],
                                 start=(dc == 0), stop=(dc == NDC - 1))
            st = small.tile([P, BLOC], F32, tag=f"sT{icc}")
            nc.vector.tensor_scalar(out=st, in0=sp, scalar1=C0,
                                    scalar2=lb_cols[:, icc:icc + 1],
                                    op0=MUL, op1=ADD)
            s2 = small.tile([P, BLOC], F32, tag=f"s2T{icc}")
            nc.vector.tensor_mul(s2, st, st)
            sT.append(st)
            s2T.append(s2)

        # ---- staging: x band -> scaled f32 xt -> V planes (bf16) ----
        vts = {}     # gb -> [vt per icc]
        nbs = {}     # gb -> [128, RB*W] f32 noise broadcast

        def stage_band(gb):
            b, band = divmod(gb, NBAND)
            slot = gb % 3
            r0 = band * RB
            vt_l = []
            for icc in range(NIC):
                xt = xt_pool.tile([P, XR, W], F32, tag="xt", name=f"xt{gb}_{icc}")
                # xt row j holds x row (r0-1+j); rows outside [0,H) are zero
                lo = max(r0 - 1, 0)
                hi = min(r0 - 1 + XR, H)
                j0 = lo - (r0 - 1)
                if j0 > 0:
                    nc.vector.memset(xt[:, 0:j0, :], 0.0)
                if (r0 - 1 + XR) > H:
                    nc.vector.memset(xt[:, XR - 1:XR, :], 0.0)
                nc.sync.dma_start(
                    out=xt[:, j0:j0 + (hi - lo), :],
                    in_=x_d[b, icc * P:(icc + 1) * P, lo:hi, :])
                # scale by s in place (ScalarE)
                nc.scalar.activation(out=xt, in_=xt, func=COPYF,
                                     scale=sT[icc][:, b:b + 1])
                # V planes: [128, 4, XR, WT] bf16
                vt = v_pool.tile([P, 4, XR, WT], BF16, tag=f"v{icc}s{slot}",
                                 name=f"v{gb}_{icc}")
                xte = xt.rearrange("p r (w two) -> p r w two", two=2)
                ev = xte[:, :, :, 0]   # x cols 0,2,..,62
                od = xte[:, :, :, 1]   # x cols 1,3,..,63
                # V0[k] = d(2k-1) - d(2k+1);  k=0: -x[1]
                nc.vector.tensor_tensor(out=vt[:, 0, :, 1:WT], in0=od[:, :, 0:WT - 1],
                                        in1=od[:, :, 1:WT], op=SUB)
                nc.gpsimd.tensor_scalar_mul(out=vt[:, 0, :, 0:1], in0=od[:, :, 0:1],
                                            scalar1=-1.0)
                # V1[k] = d(2k) + d(2k+1)
                nc.vector.tensor_tensor(out=vt[:, 1], in0=ev, in1=od, op=ADD)
                # V2[k] = d(2k+1) - d(2k)
                nc.gpsimd.tensor_tensor(out=vt[:, 2], in0=od, in1=ev, op=SUB)
                # V3[k] = d(2k) - d(2k+2);  k=WT-1: x[62]
                nc.gpsimd.tensor_tensor(out=vt[:, 3, :, 0:WT - 1], in0=ev[:, :, 0:WT - 1],
                                        in1=ev[:, :, 1:WT], op=SUB)
                nc.gpsimd.tensor_copy(out=vt[:, 3, :, WT - 1:WT], in_=ev[:, :, WT - 1:WT])
                vt_l.append(vt)
            vts[gb] = vt_l
            # noise rows broadcast to all partitions (partition step-0 src AP)
            nb = nb_pool.tile([P, RB * W], F32, tag=f"nb{slot}", name=f"nb{gb}")
            nsrc = noise_d[b].rearrange("o h w -> o (h w)")[0:1, r0 * W:(r0 + RB) * W]
            nsrc_bc = bass.AP(tensor=nsrc.tensor, offset=nsrc.offset,
                              ap=[[0, P]] + list(nsrc.ap)[1:])
            nc.scalar.dma_start(out=nb, in_=nsrc_bc)
            nbs[gb] = nb

        # ---- weights: wchunk[ic, tap, oc] (halved), u1/u2, sqw/wsq ----
        wchunks = {}
        u1s = {}
        u2s = {}
        wsqs = {}
        sig_scale = {}   # occ -> [P, BLOC] f32: C1*sigma_inv

        def emit_weights(occ):
            for icc in range(NIC):
                co = co_tiles.pop((icc, occ))
                # cast to bf16 in tap-major order with the Winograd 1/2
                # folded in (strided read, contiguous write)
                co_bf = cobf_pool.tile([P, KK * KK * P], BF16, tag="cobf")
                nc.scalar.activation(
                    out=co_bf.rearrange("o (n i) -> o n i", i=P),
                    in_=co.rearrange("o (i n) -> o n i", n=KK * KK),
                    func=COPYF, scale=0.5)
                # one batched xbar transpose: wchunk[ic, tap, oc]
                wchunk = wt_pool.tile([P, KK * KK, P], BF16,
                                      tag=f"wt{icc}_{occ}", name=f"wt{icc}_{occ}")
                nc.sync.dma_start_transpose(out=wchunk, in_=co_bf)
                wchunks[(icc, occ)] = wchunk
                # U1/U2 = (g0 +- g1 + g2)/2 of the true taps (halved taps add)
                wc4 = wchunk.rearrange("i (ky kx) o -> i ky kx o", kx=KK)
                w0 = wc4[:, :, 0, :]
                w1 = wc4[:, :, 1, :]
                w2 = wc4[:, :, 2, :]
                ua = ua_pool.tile([P, KK, P], BF16, tag="ua")
                nc.vector.tensor_tensor(out=ua, in0=w0, in1=w2, op=ADD)
                u1 = u_pool.tile([P, KK, P], BF16, tag=f"u1_{icc}_{occ}",
                                 name=f"u1_{icc}_{occ}")
                nc.vector.tensor_tensor(out=u1, in0=ua, in1=w1, op=ADD)
                u2 = u_pool.tile([P, KK, P], BF16, tag=f"u2_{icc}_{occ}",
                                 name=f"u2_{icc}_{occ}")
                nc.gpsimd.tensor_tensor(out=u2, in0=ua, in1=w1, op=SUB)
                u1s[(icc, occ)] = u1
                u2s[(icc, occ)] = u2
                # sigma prep: sqw = wchunk^2 (0.25*w^2), wsq = sum over taps
                sqw = sqw_pool.tile([P, KK * KK, P], BF16, tag="sqw")
                nc.gpsimd.tensor_tensor(out=sqw, in0=wchunk, in1=wchunk, op=MUL)
                wsq = wsq_pool.tile([P, P], F32, tag=f"wsq{icc}", name=f"wsq{icc}_{occ}")
                nc.vector.tensor_reduce(
                    out=wsq, in_=sqw.rearrange("i t o -> i o t"),
                    axis=mybir.AxisListType.X, op=ADD)
                wsqs[(icc, occ)] = wsq
            if occ + 1 < NOC:
                load_co(occ + 1)

        def emit_sigma(occ):
            sg = psum.tile([P, BLOC], F32, tag="mm", name=f"sg{occ}")
            for icc in range(NIC):
                nc.tensor.matmul(sg, wsqs.pop((icc, occ)), s2T[icc],
                                 start=(icc == 0), stop=(icc == NIC - 1))
            # sg = 0.25 * sum w^2 s^2; sig = sqrt(C1^2 * 4 * sg + EPS)
            sig = small.tile([P, BLOC], F32, tag=f"sig{occ}")
            nc.scalar.activation(out=sig, in_=sg, func=SQRTF,
                                 bias=eps_col[:, 0:1], scale=4.0 * C1 * C1)
            sinv = small.tile([P, BLOC], F32, tag=f"sinv{occ}")
            nc.vector.reciprocal(out=sinv, in_=sig)
            ssc = small.tile([P, BLOC], F32, tag=f"ssc{occ}")
            nc.gpsimd.tensor_scalar_mul(out=ssc, in0=sinv, scalar1=C1)
            sig_scale[occ] = ssc

        # ---- conv group + post ----
        out3 = out_d.rearrange("b c h w -> b c (h w)")

        def group(b, band, occ):
            gb = b * NBAND + band
            vt_l = vts[gb]
            nb_t = nbs[gb]
            M = [psum.tile([P, RB * WT], F32, tag="mm", name=f"M{xi}")
                 for xi in range(4)]
            for icc in range(NIC):
                vt = vt_l[icc]
                wc = wchunks[(icc, occ)]
                for ky in range(KK):
                    lhs = (wc[:, 3 * ky, :],
                           u1s[(icc, occ)][:, ky, :],
                           u2s[(icc, occ)][:, ky, :],
                           wc[:, 3 * ky + 2, :])
                    st = (icc == 0 and ky == 0)
                    sp = (icc == NIC - 1 and ky == KK - 1)
                    for xi in range(4):
                        nc.tensor.matmul(
                            M[xi], lhs[xi], vt[:, xi, ky:ky + RB, :],
                            start=st, stop=sp)
            # drains: one PSUM operand per op.  ScalarE: m1 = M1 -> SBUF.
            m1 = dr_pool.tile([P, RB * WT], F32, tag="m1", name="m1")
            nc.scalar.activation(out=m1, in_=M[1], func=COPYF)
            # DVE: t = 2*M0 + m1 ; e = t + M2 ; t2 = m1 - M2 ; o = -2*M3 + t2
            t = dr_pool.tile([P, RB * WT], F32, tag="t", name="t", bufs=1)
            nc.vector.scalar_tensor_tensor(out=t, in0=M[0], scalar=2.0,
                                           in1=m1, op0=MUL, op1=ADD)
            e = dr_pool.tile([P, RB * WT], F32, tag="e", name="e")
            nc.vector.tensor_tensor(out=e, in0=t, in1=M[2], op=ADD)
            t2 = dr_pool.tile([P, RB * WT], F32, tag="t2", name="t2", bufs=1)
            nc.vector.tensor_tensor(out=t2, in0=m1, in1=M[2], op=SUB)
            o = dr_pool.tile([P, RB * WT], F32, tag="o", name="o")
            nc.vector.scalar_tensor_tensor(out=o, in0=M[3], scalar=-2.0,
                                           in1=t2, op0=MUL, op1=ADD)
            # ScalarE: pre = sn*noise + bias ; DVE: z2 = ssc*e/o + pre
            ssc = sig_scale[occ][:, b:b + 1]
            pre = pz_pool.tile([P, RB, W], BF16, tag="pre", name="pre")
            nc.scalar.activation(out=pre,
                                 in_=nb_t.rearrange("p (r w) -> p r w", r=RB),
                                 func=mybir.ActivationFunctionType.Identity,
                                 scale=sn_cols[:, occ:occ + 1],
                                 bias=bias_cols[:, occ:occ + 1])
            prev = pre.rearrange("p r (w two) -> p r w two", two=2)
            ev3 = e.rearrange("p (r w) -> p r w", w=WT)
            ov3 = o.rearrange("p (r w) -> p r w", w=WT)
            z2e = pz_pool.tile([P, RB, WT], BF16, tag="z2e", name="z2e")
            nc.vector.scalar_tensor_tensor(out=z2e, in0=ev3, scalar=ssc,
                                           in1=prev[:, :, :, 0], op0=MUL, op1=ADD)
            z2o = pz_pool.tile([P, RB, WT], BF16, tag="z2o", name="z2o")
            nc.vector.scalar_tensor_tensor(out=z2o, in0=ov3, scalar=ssc,
                                           in1=prev[:, :, :, 1], op0=MUL, op1=ADD)
            # DVE: leaky relu, interleaving even/odd back to row-major
            zo = out_pool.tile([P, RB, WT, 2], F32, tag="zo", name="zo")
            nc.vector.scalar_tensor_tensor(out=zo[:, :, :, 0], in0=z2e, scalar=SLOPE,
                                           in1=z2e, op0=MUL, op1=MAX)
            nc.vector.scalar_tensor_tensor(out=zo[:, :, :, 1], in0=z2o, scalar=SLOPE,
                                           in1=z2o, op0=MUL, op1=MAX)
            nc.sync.dma_start(
                out=out3[b, occ * P:(occ + 1) * P,
                         band * RB * W:(band + 1) * RB * W],
                in_=zo.rearrange("p r w two -> p (r w two)"))

        # ---- prologue staging/emission ----
        stage_band(0)
        emit_weights(0)
        stage_band(1)

        # ---- main loop: occ-outer over band pairs ----
        for b in range(BLOC):
            for pp in range(NBAND // 2):
                bnd0 = 2 * pp
                g0 = b * NBAND + bnd0
                for occ in range(NOC):
                    if b == 0 and pp == 0:
                        emit_sigma(occ)
                    group(b, bnd0, occ)
                    if b == 0 and pp == 0 and occ + 1 < NOC:
                        emit_weights(occ + 1)
                    if occ == 1 and g0 + 2 < BLOC * NBAND:
                        stage_band(g0 + 2)
                    if occ == 3 and g0 + 3 < BLOC * NBAND:
                        stage_band(g0 + 3)
                    group(b, bnd0 + 1, occ)
                del vts[g0], vts[g0 + 1], nbs[g0], nbs[g0 + 1]

    nc.compile()
    return nc


_NC_CACHE = None


def _get_nc():
    global _NC_CACHE
    if _NC_CACHE is None:
        _NC_CACHE = build_nc()
    return _NC_CACHE


def kernel(**inputs):
    from concourse.bass_utils import run_bass_kernel_spmd

    nc = _get_nc()
    shard_names = ("x", "w", "noise")
    in_maps = []
    for i in range(NCORES):
        m = {}
        for k, v in inputs.items():
            v = np.ascontiguousarray(np.asarray(v), dtype=np.float32)
            if k in shard_names:
                m[k] = np.ascontiguousarray(v[i * BLOC:(i + 1) * BLOC])
            else:
                m[k] = v
        in_maps.append(m)
    res = run_bass_kernel_spmd(nc, in_maps, list(range(NCORES)))
    outs = [res.results[i]["out"] for i in range(NCORES)]
    return np.concatenate(outs, axis=0).astype(np.float32)
